# revision 23
# baseline (speedup 1.0000x reference)
"""Trainium2 Bass kernel for the BF16Indexer sparse-attention problem.

Computes, for B=1, M=2048, H=32, D=128, N=4096:
    logits = einsum('bmhd,bnd->bmhn', q, k)          (fp32 accum)
    o      = einsum('bmhn,bmh->bmn', relu(logits), w) / sqrt(D)

Sharding: M (query tokens) split across 8 cores; k replicated.

Per-core algorithm (M_loc = 256 rows, mh = M_loc*H = 8192):
  - qT  [128=d, mh]     (host-transposed shard of q)
  - kT  [128=d, N]      (host-transposed k, replicated)
  - wblk[128, n_tiles*128]  block-diagonal per-tile weight matrices
  - mm1 (PE):  for each mh-tile t (128 rows = 4 m's x 32 h):
        p1 = qT[:, t].T @ kT[:, chunk]         -> logits [128, 512] fp32 PSUM
  - drain (ACT on even tiles / DVE on odd): y = relu(scale*p1) -> bf16 SBUF
  - mm2 (PE):  p2[:, chunk] += wblk[:, t].T @ y  accumulated over the 32
        tiles of a group (block-diagonal lhsT routes each tile's 4 m's to
        the right 4 of 128 output partitions)
  - p2 [128=m, n_chunk] fp32 -> SBUF -> DMA to o[m, n]

The whole kernel is one flat software pipeline over (group, n-half, tile)
with mm2 trailing mm1 by DELAY tiles, so the PE streams matmuls
back-to-back (~215ns each) across pass boundaries. Steady state is
PE-bound at ~128 elem/cycle ingest for both matmuls (~220us/core); the
PSUM->SBUF relu drains run concurrently on ACT+DVE (~69% busy each).
PE warm-up matmuls trip the HAM clock gate to 2.4GHz during the initial
DMA loads.

kernel(**inputs) takes the FULL inputs and returns the FULL (1, 2048, 4096)
fp32 output; sharding/gather is host-side marshalling only (no host FLOPs).
Measured: ~242us HW exec per core (8 cores SPMD, PE 92% busy),
rel err 1.8e-3.
"""

import math
import numpy as np
import ml_dtypes

import concourse.bass as bass
import concourse.mybir as mybir
import concourse.tile as tile
from concourse import bacc
from concourse.bass_utils import run_bass_kernel_spmd

# Problem constants (hardcoded per harness contract)
B, M, H, D, N = 1, 2048, 32, 128, 4096
N_CORES = 8
M_LOC = M // N_CORES              # 256 query rows per core
MH = M_LOC * H                    # 8192
N_TILES = MH // 128               # 64 mh-tiles (4 m's each)
SOFTMAX_SCALE = 1.0 / math.sqrt(float(D))


def build_nc(m_loc=M_LOC, n=N, group_tiles=32, n_chunk=1024):
    """Build + compile the per-core bass program.

    group_tiles: mh-tiles per mm2 accumulation group (psum2 has
                 4*group_tiles output partitions).
    n_chunk:     n-columns processed per (group, half) pass; psum2 is
                 [128, n_chunk] fp32 = n_chunk/512 PSUM banks.
    """
    mh = m_loc * H
    n_tiles = mh // 128
    assert n_tiles % group_tiles == 0
    n_groups = n_tiles // group_tiles
    assert n % n_chunk == 0
    n_halves = n // n_chunk
    assert n_chunk % 512 == 0
    c_per_half = n_chunk // 512
    gp = 4 * group_tiles  # output partitions per group

    nc = bacc.Bacc("TRN2", target_bir_lowering=False, debug=False)

    bf16 = mybir.dt.bfloat16
    f32 = mybir.dt.float32

    qT_d = nc.dram_tensor("qT", [128, mh], bf16, kind="ExternalInput")
    kT_d = nc.dram_tensor("kT", [128, n], bf16, kind="ExternalInput")
    wblk_d = nc.dram_tensor("wblk", [128, n_tiles * gp], bf16, kind="ExternalInput")
    f8 = mybir.dt.float8e4
    S_d = nc.dram_tensor("S", [128, n_groups * 8 * 256], f8, kind="ExternalInput")
    o_d = nc.dram_tensor("o", [m_loc, n], f32, kind="ExternalOutput")

    with tile.TileContext(nc) as tc:
        with (
            tc.tile_pool(name="const", bufs=1) as const_pool,
            tc.tile_pool(name="ypool", bufs=5) as ypool,
            tc.tile_pool(name="psum1", bufs=6, space="PSUM") as psum1,
            tc.tile_pool(name="psum2", bufs=2, space="PSUM") as psum2,
            tc.tile_pool(name="ostage", bufs=4) as ostage,
        ):
            qT = const_pool.tile([128, mh], bf16)
            kT = const_pool.tile([128, n], bf16)
            wblk = const_pool.tile([128, n_tiles * gp], bf16)
            S = const_pool.tile([128, n_groups * 8 * 256], f8)

            def S_ap(g, i):
                return S[:, bass.ds((g * 8 + i) * 256, 256)].rearrange(
                    "p (j m) -> p j m", j=2)

            pair_tiles = {}

            wb_n = n_tiles * gp
            # gpsimd's SWDGE queue measured ~5x faster than the sync/scalar
            # HW queues: give it the whole latency-critical early set
            # (qT-g0, kT[0:2048], S-g0); wblk spreads over sync+scalar.
            nc.gpsimd.dma_start(qT[:, 0:256], qT_d[:, 0:256])
            nc.gpsimd.dma_start(kT[:, :512], kT_d[:, :512])
            nc.gpsimd.dma_start(kT[:, 512:1024], kT_d[:, 512:1024])
            nc.gpsimd.dma_start(qT[:, 256:1024], qT_d[:, 256:1024])
            nc.gpsimd.dma_start(wblk[:, 0:1024], wblk_d[:, 0:1024])
            nc.gpsimd.dma_start(S[:, :2048], S_d[:, :2048])
            nc.gpsimd.dma_start(kT[:, 1024:2048], kT_d[:, 1024:2048])
            nc.gpsimd.dma_start(wblk[:, 1024:2048], wblk_d[:, 1024:2048])
            nc.gpsimd.dma_start(qT[:, 1024:2048], qT_d[:, 1024:2048])
            nc.gpsimd.dma_start(qT[:, 2048:3072], qT_d[:, 2048:3072])
            nc.gpsimd.dma_start(qT[:, 3072:4096], qT_d[:, 3072:4096])
            nc.gpsimd.dma_start(qT[:, 4096:mh], qT_d[:, 4096:mh])
            nc.sync.dma_start(kT[:, 2048:3072], kT_d[:, 2048:3072])
            nc.sync.dma_start(S[:, 2048:], S_d[:, 2048:])
            nc.scalar.dma_start(kT[:, 3072:4096], kT_d[:, 3072:4096])
            nc.scalar.dma_start(wblk[:, 4096:6144], wblk_d[:, 4096:6144])
            # warm the ACT spline tables while DMAs run
            warm = const_pool.tile([128, 1], bf16)
            nc.gpsimd.memset(warm[:], 0)
            nc.scalar.activation(warm[:], warm[:],
                                 mybir.ActivationFunctionType.Relu)

            # warm the PE (HAM un-throttles after ~3.4us of activity) with
            # small matmuls on a zeroed scratch tile while DMAs run
            if n_tiles >= 16:
                wsrc = const_pool.tile([128, 128], bf16)
                nc.gpsimd.memset(wsrc[:], 0)
                wps = psum1.tile([128, 128], f32, tag="p1", name="warm_ps")
                for _ in range(52):
                    nc.tensor.matmul(wps[:], wsrc[:], wsrc[:],
                                     start=True, stop=True)

            def emit_mm1(g, hf, t):
                """mm1 for one mh-tile: c_per_half [128,512] psum tiles, each
                drained (relu+scale -> bf16) on a fixed engine per chunk."""
                tg = g * group_tiles + t
                qT_t = qT[:, bass.ts(tg, 128)]
                small = t >= 16
                if small:
                    i, j = divmod(t - 16, 2)
                    if j == 0:
                        pair_tiles[(g, hf, i)] = ypool.tile(
                            [128, 2, n_chunk], mybir.dt.float8e4, tag="yp",
                            name=f"yp_{g}_{hf}_{i}")
                    y_t = pair_tiles[(g, hf, i)]
                else:
                    y_t = ypool.tile([128, n_chunk], bf16, tag="y")
                for c in range(c_per_half):
                    p1 = psum1.tile([128, 512], f32)
                    nc.tensor.matmul(
                        p1[:],
                        qT_t,
                        kT[:, bass.ds(hf * n_chunk + c * 512, 512)],
                        start=True,
                        stop=True,
                    )
                    if small:
                        ysl = y_t[:, j, bass.ds(c * 512, 512)]
                    else:
                        ysl = y_t[:, bass.ts(c, 512)]
                    if t % 2 == 0:
                        nc.scalar.activation(
                            ysl, p1[:],
                            mybir.ActivationFunctionType.Relu,
                            scale=SOFTMAX_SCALE,
                        )
                    else:
                        nc.vector.tensor_scalar(
                            ysl, p1[:], SOFTMAX_SCALE, 0.0,
                            mybir.AluOpType.mult, mybir.AluOpType.max,
                        )
                return y_t

            def emit_mm2(p2_chunks, g, hf, t, y_t):
                if t >= 16:
                    i, j = divmod(t - 16, 2)
                    if j == 0:
                        return  # partner tile completes the pair
                    yp = pair_tiles.pop((g, hf, i))
                    for c in range(c_per_half):
                        nc.tensor.matmul(
                            p2_chunks[c][:],
                            S_ap(g, i),
                            yp[:, :, bass.ds(c * 512, 512)],
                            start=False,
                            stop=(t == group_tiles - 1),
                            perf_mode=mybir.MatmulPerfMode.DoubleRow,
                        )
                    return
                tg = g * group_tiles + t
                w_t = wblk[:, bass.ts(tg, gp)]
                for c in range(c_per_half):
                    nc.tensor.matmul(
                        p2_chunks[c][:],
                        w_t,
                        y_t[:, bass.ts(c, 512)],
                        start=(t == 0),
                        stop=False,
                    )

            DELAY = 3  # tiles of run-ahead before mm2 consumes a drained y

            def finish_pass(g, hf, p2_chunks):
                # per-chunk psum2 drain, alternating engines; stores on
                # two queues so the final store isn't one long DMA
                for c in range(c_per_half):
                    ost = ostage.tile([gp, 512], f32, tag="ost",
                                      name=f"ost_{g}_{hf}_{c}")
                    if (hf * c_per_half + c) % 2 == 0:
                        nc.vector.tensor_copy(ost[:], p2_chunks[c][:])
                    else:
                        nc.scalar.copy(ost[:], p2_chunks[c][:])
                    (nc.sync if c % 2 == 0 else nc.scalar).dma_start(
                        o_d[bass.ts(g, gp),
                            bass.ds(hf * n_chunk + c * 512, 512)],
                        ost[:],
                    )

            # Flat tile stream across all (group, half) passes with mm2
            # trailing DELAY tiles behind mm1 — the pipeline crosses pass
            # boundaries so the PE never drains at a boundary.
            passes = [(g, hf) for g in range(n_groups) for hf in range(n_halves)]
            stream = [(pi, t) for pi in range(len(passes))
                      for t in range(group_tiles)]
            p2_of = {}
            ys = {}
            for idx, (pi, t) in enumerate(stream):
                g, hf = passes[pi]
                ys[idx] = emit_mm1(g, hf, t)
                j = idx - DELAY
                if j >= 0:
                    pj, tj = stream[j]
                    gj, hfj = passes[pj]
                    if pj not in p2_of:
                        p2_of[pj] = [
                            psum2.tile([gp, 512], f32, tag="p2",
                                       name=f"p2_{gj}_{hfj}_{c}")
                            for c in range(c_per_half)
                        ]
                    emit_mm2(p2_of[pj], gj, hfj, tj, ys.pop(j))
                    if tj == group_tiles - 1:
                        finish_pass(gj, hfj, p2_of.pop(pj))
            for j in range(len(stream) - DELAY, len(stream)):
                pj, tj = stream[j]
                gj, hfj = passes[pj]
                if pj not in p2_of:
                    p2_of[pj] = [
                        psum2.tile([gp, 512], f32, tag="p2",
                                   name=f"p2_{gj}_{hfj}_{c}")
                        for c in range(c_per_half)
                    ]
                emit_mm2(p2_of[pj], gj, hfj, tj, ys.pop(j))
                if tj == group_tiles - 1:
                    finish_pass(gj, hfj, p2_of.pop(pj))

    nc.compile()
    return nc


def marshal_core_inputs(q, k, weights, core, m_loc=M_LOC, group_tiles=32):
    """Host-side layout marshalling for one core (permute/transpose/cast).

    Tile layout: 8 m's x 16 head-ranks per 128-col tile. Per group of
    128 m's: tiles 0..15 carry each m-octet's 16 largest-|w| heads
    (bf16 path, w in wblk); tiles 16..31 carry the 16 smallest (fp8
    path, w in the DoubleRow S blocks)."""
    f8 = ml_dtypes.float8_e4m3
    bf16 = ml_dtypes.bfloat16

    q_sh = np.asarray(q[0, core * m_loc:(core + 1) * m_loc])   # (256,H,D) bf16
    w_sh = np.asarray(weights[core * m_loc:(core + 1) * m_loc, 0, :]).astype(np.float32)
    order = np.argsort(-np.abs(w_sh), axis=1)                  # (256,H)
    w_sorted = np.take_along_axis(w_sh, order, 1)              # (256,H)
    q_sorted = np.take_along_axis(q_sh, order[:, :, None], 1)  # (256,H,D)

    kT = np.ascontiguousarray(np.asarray(k[0]).T)              # (128,N)

    gp = 4 * group_tiles  # 128
    qT = np.empty((128, MH), dtype=bf16)
    wblk = np.zeros((128, N_TILES * gp), dtype=bf16)
    S = np.zeros((128, 2 * 8 * 2 * 128), dtype=f8)
    rows = np.arange(128)
    for g in range(2):
        for t in range(32):
            tg = g * 32 + t
            o = t % 16                              # m-octet within group
            r0 = 0 if t < 16 else 16                # head-rank offset
            ms = 128 * g + 8 * o + np.arange(8)     # the 8 m's (global)
            blk = q_sorted[ms][:, r0:r0 + 16]       # (8,16,D)
            qT[:, tg * 128:(tg + 1) * 128] = blk.reshape(128, D).T
            wv = w_sorted[ms][:, r0:r0 + 16].reshape(128)   # p = 16*mi + r
            cols = np.repeat(8 * o + np.arange(8), 16)      # local m per p
            if t < 16:
                wblk[rows, tg * gp + cols] = wv.astype(bf16)
            else:
                i, j = divmod(t - 16, 2)
                S[rows, ((g * 8 + i) * 2 + j) * 128 + cols] = wv.astype(f8)

    return {"qT": qT, "kT": kT, "wblk": wblk, "S": S}


_NC_CACHE = {}


def _get_nc():
    if "nc" not in _NC_CACHE:
        _NC_CACHE["nc"] = build_nc()
    return _NC_CACHE["nc"]


def kernel(q, k, weights):
    nc = _get_nc()
    in_maps = [marshal_core_inputs(q, k, weights, c) for c in range(N_CORES)]
    res = run_bass_kernel_spmd(nc, in_maps, list(range(N_CORES)))
    out = np.concatenate([res.results[c]["o"] for c in range(N_CORES)], axis=0)
    return out[None]  # (1, M, N) fp32



# revision 25
# speedup vs baseline: 1.0102x; 1.0102x over previous
"""Trainium2 Bass kernel for the BF16Indexer sparse-attention problem.

Computes, for B=1, M=2048, H=32, D=128, N=4096:
    logits = einsum('bmhd,bnd->bmhn', q, k)          (fp32 accum)
    o      = einsum('bmhn,bmh->bmn', relu(logits), w) / sqrt(D)

Sharding: M (query tokens) split across 8 cores; k replicated.

Per-core algorithm (M_loc = 256 rows, mh = M_loc*H = 8192):
  - qT  [128=d, mh]     (host-transposed shard of q)
  - kT  [128=d, N]      (host-transposed k, replicated)
  - wblk[128, n_tiles*128]  block-diagonal per-tile weight matrices
  - mm1 (PE):  for each mh-tile t (128 rows = 4 m's x 32 h):
        p1 = qT[:, t].T @ kT[:, chunk]         -> logits [128, 512] fp32 PSUM
  - drain (ACT on even tiles / DVE on odd): y = relu(scale*p1) -> bf16 SBUF
  - mm2 (PE):  p2[:, chunk] += wblk[:, t].T @ y  accumulated over the 32
        tiles of a group (block-diagonal lhsT routes each tile's 4 m's to
        the right 4 of 128 output partitions)
  - p2 [128=m, n_chunk] fp32 -> SBUF -> DMA to o[m, n]

The whole kernel is one flat software pipeline over (group, n-half, tile)
with mm2 trailing mm1 by DELAY tiles, so the PE streams matmuls
back-to-back (~215ns each) across pass boundaries. Steady state is
PE-bound at ~128 elem/cycle ingest for both matmuls (~220us/core); the
PSUM->SBUF relu drains run concurrently on ACT+DVE (~69% busy each).
PE warm-up matmuls trip the HAM clock gate to 2.4GHz during the initial
DMA loads.

kernel(**inputs) takes the FULL inputs and returns the FULL (1, 2048, 4096)
fp32 output; sharding/gather is host-side marshalling only (no host FLOPs).
Measured: ~242us HW exec per core (8 cores SPMD, PE 92% busy),
rel err 1.8e-3.
"""

import math
import numpy as np
import ml_dtypes

import concourse.bass as bass
import concourse.mybir as mybir
import concourse.tile as tile
from concourse import bacc
from concourse.bass_utils import run_bass_kernel_spmd

# Problem constants (hardcoded per harness contract)
B, M, H, D, N = 1, 2048, 32, 128, 4096
N_CORES = 8
M_LOC = M // N_CORES              # 256 query rows per core
MH = M_LOC * H                    # 8192
N_TILES = MH // 128               # 64 mh-tiles (4 m's each)
SOFTMAX_SCALE = 1.0 / math.sqrt(float(D))


def build_nc(m_loc=M_LOC, n=N, group_tiles=32, n_chunk=1024):
    """Build + compile the per-core bass program.

    group_tiles: mh-tiles per mm2 accumulation group (psum2 has
                 4*group_tiles output partitions).
    n_chunk:     n-columns processed per (group, half) pass; psum2 is
                 [128, n_chunk] fp32 = n_chunk/512 PSUM banks.
    """
    mh = m_loc * H
    n_tiles = mh // 128
    assert n_tiles % group_tiles == 0
    n_groups = n_tiles // group_tiles
    assert n % n_chunk == 0
    n_halves = n // n_chunk
    assert n_chunk % 512 == 0
    c_per_half = n_chunk // 512
    gp = 4 * group_tiles  # output partitions per group

    nc = bacc.Bacc("TRN2", target_bir_lowering=False, debug=False)

    bf16 = mybir.dt.bfloat16
    f32 = mybir.dt.float32

    qT_d = nc.dram_tensor("qT", [128, mh], bf16, kind="ExternalInput")
    kT_d = nc.dram_tensor("kT", [128, n], bf16, kind="ExternalInput")
    wblk_d = nc.dram_tensor("wblk", [128, n_tiles * gp], bf16, kind="ExternalInput")
    f8 = mybir.dt.float8e4
    S_d = nc.dram_tensor("S", [128, n_groups * 8 * 256], f8, kind="ExternalInput")
    o_d = nc.dram_tensor("o", [m_loc, n], f32, kind="ExternalOutput")

    with tile.TileContext(nc) as tc:
        with (
            tc.tile_pool(name="const", bufs=1) as const_pool,
            tc.tile_pool(name="ypool", bufs=5) as ypool,
            tc.tile_pool(name="psum1", bufs=6, space="PSUM") as psum1,
            tc.tile_pool(name="psum2", bufs=2, space="PSUM") as psum2,
            tc.tile_pool(name="ostage", bufs=4) as ostage,
        ):
            qT = const_pool.tile([128, mh], bf16)
            kT = const_pool.tile([128, n], bf16)
            wblk = const_pool.tile([128, n_tiles * gp], bf16)
            S = const_pool.tile([128, n_groups * 8 * 256], f8)

            def S_ap(g, i):
                return S[:, bass.ds((g * 8 + i) * 256, 256)].rearrange(
                    "p (j m) -> p j m", j=2)

            pair_tiles = {}

            wb_n = n_tiles * gp
            # gpsimd's SWDGE queue measured ~5x faster than the sync/scalar
            # HW queues: give it the whole latency-critical early set
            # (qT-g0, kT[0:2048], S-g0); wblk spreads over sync+scalar.
            nc.gpsimd.dma_start(qT[:, 0:256], qT_d[:, 0:256])
            nc.gpsimd.dma_start(kT[:, :512], kT_d[:, :512])
            nc.gpsimd.dma_start(kT[:, 512:1024], kT_d[:, 512:1024])
            nc.gpsimd.dma_start(qT[:, 256:1024], qT_d[:, 256:1024])
            nc.gpsimd.dma_start(kT[:, 1024:2048], kT_d[:, 1024:2048])
            nc.gpsimd.dma_start(qT[:, 1024:2048], qT_d[:, 1024:2048])
            nc.gpsimd.dma_start(qT[:, 2048:3072], qT_d[:, 2048:3072])
            nc.gpsimd.dma_start(S[:, :2048], S_d[:, :2048])
            nc.gpsimd.dma_start(qT[:, 3072:4096], qT_d[:, 3072:4096])
            nc.gpsimd.dma_start(qT[:, 4096:mh], qT_d[:, 4096:mh])
            nc.sync.dma_start(wblk[:, 0:256], wblk_d[:, 0:256])
            nc.sync.dma_start(wblk[:, 256:1024], wblk_d[:, 256:1024])
            nc.sync.dma_start(kT[:, 2048:3072], kT_d[:, 2048:3072])
            nc.sync.dma_start(S[:, 2048:], S_d[:, 2048:])
            nc.scalar.dma_start(wblk[:, 1024:2048], wblk_d[:, 1024:2048])
            nc.scalar.dma_start(kT[:, 3072:4096], kT_d[:, 3072:4096])
            nc.scalar.dma_start(wblk[:, 4096:6144], wblk_d[:, 4096:6144])
            # warm the ACT spline tables while DMAs run
            warm = const_pool.tile([128, 1], bf16)
            nc.gpsimd.memset(warm[:], 0)
            nc.scalar.activation(warm[:], warm[:],
                                 mybir.ActivationFunctionType.Relu)

            # warm the PE (HAM un-throttles after ~3.4us of activity) with
            # small matmuls on a zeroed scratch tile while DMAs run
            if n_tiles >= 16:
                wsrc = const_pool.tile([128, 128], bf16)
                nc.gpsimd.memset(wsrc[:], 0)
                wps = psum1.tile([128, 128], f32, tag="p1", name="warm_ps")
                for _ in range(52):
                    nc.tensor.matmul(wps[:], wsrc[:], wsrc[:],
                                     start=True, stop=True)

            def emit_mm1(g, hf, t):
                """mm1 for one mh-tile: c_per_half [128,512] psum tiles, each
                drained (relu+scale -> bf16) on a fixed engine per chunk."""
                tg = g * group_tiles + t
                qT_t = qT[:, bass.ts(tg, 128)]
                small = t >= 16
                if small:
                    i, j = divmod(t - 16, 2)
                    if j == 0:
                        pair_tiles[(g, hf, i)] = ypool.tile(
                            [128, 2, n_chunk], mybir.dt.float8e4, tag="yp",
                            name=f"yp_{g}_{hf}_{i}")
                    y_t = pair_tiles[(g, hf, i)]
                else:
                    y_t = ypool.tile([128, n_chunk], bf16, tag="y")
                for c in range(c_per_half):
                    p1 = psum1.tile([128, 512], f32)
                    nc.tensor.matmul(
                        p1[:],
                        qT_t,
                        kT[:, bass.ds(hf * n_chunk + c * 512, 512)],
                        start=True,
                        stop=True,
                    )
                    if small:
                        ysl = y_t[:, j, bass.ds(c * 512, 512)]
                    else:
                        ysl = y_t[:, bass.ts(c, 512)]
                    if t % 2 == 0:
                        nc.scalar.activation(
                            ysl, p1[:],
                            mybir.ActivationFunctionType.Relu,
                            scale=SOFTMAX_SCALE,
                        )
                    else:
                        nc.vector.tensor_scalar(
                            ysl, p1[:], SOFTMAX_SCALE, 0.0,
                            mybir.AluOpType.mult, mybir.AluOpType.max,
                        )
                return y_t

            def emit_mm2(p2_chunks, g, hf, t, y_t):
                if t >= 16:
                    i, j = divmod(t - 16, 2)
                    if j == 0:
                        return  # partner tile completes the pair
                    yp = pair_tiles.pop((g, hf, i))
                    for c in range(c_per_half):
                        nc.tensor.matmul(
                            p2_chunks[c][:],
                            S_ap(g, i),
                            yp[:, :, bass.ds(c * 512, 512)],
                            start=False,
                            stop=(t == group_tiles - 1),
                            perf_mode=mybir.MatmulPerfMode.DoubleRow,
                        )
                    return
                tg = g * group_tiles + t
                w_t = wblk[:, bass.ts(tg, gp)]
                for c in range(c_per_half):
                    nc.tensor.matmul(
                        p2_chunks[c][:],
                        w_t,
                        y_t[:, bass.ts(c, 512)],
                        start=(t == 0),
                        stop=False,
                    )

            DELAY = 3  # tiles of run-ahead before mm2 consumes a drained y

            def finish_pass(g, hf, p2_chunks):
                # per-chunk psum2 drain, alternating engines; stores on
                # two queues so the final store isn't one long DMA
                for c in range(c_per_half):
                    ost = ostage.tile([gp, 512], f32, tag="ost",
                                      name=f"ost_{g}_{hf}_{c}")
                    if (hf * c_per_half + c) % 2 == 0:
                        nc.vector.tensor_copy(ost[:], p2_chunks[c][:])
                    else:
                        nc.scalar.copy(ost[:], p2_chunks[c][:])
                    (nc.sync if c % 2 == 0 else nc.scalar).dma_start(
                        o_d[bass.ts(g, gp),
                            bass.ds(hf * n_chunk + c * 512, 512)],
                        ost[:],
                    )

            # Flat tile stream across all (group, half) passes with mm2
            # trailing DELAY tiles behind mm1 — the pipeline crosses pass
            # boundaries so the PE never drains at a boundary.
            passes = [(g, hf) for g in range(n_groups) for hf in range(n_halves)]
            stream = [(pi, t) for pi in range(len(passes))
                      for t in range(group_tiles)]
            p2_of = {}
            ys = {}
            for idx, (pi, t) in enumerate(stream):
                g, hf = passes[pi]
                ys[idx] = emit_mm1(g, hf, t)
                j = idx - DELAY
                if j >= 0:
                    pj, tj = stream[j]
                    gj, hfj = passes[pj]
                    if pj not in p2_of:
                        p2_of[pj] = [
                            psum2.tile([gp, 512], f32, tag="p2",
                                       name=f"p2_{gj}_{hfj}_{c}")
                            for c in range(c_per_half)
                        ]
                    emit_mm2(p2_of[pj], gj, hfj, tj, ys.pop(j))
                    if tj == group_tiles - 1:
                        finish_pass(gj, hfj, p2_of.pop(pj))
            for j in range(len(stream) - DELAY, len(stream)):
                pj, tj = stream[j]
                gj, hfj = passes[pj]
                if pj not in p2_of:
                    p2_of[pj] = [
                        psum2.tile([gp, 512], f32, tag="p2",
                                   name=f"p2_{gj}_{hfj}_{c}")
                        for c in range(c_per_half)
                    ]
                emit_mm2(p2_of[pj], gj, hfj, tj, ys.pop(j))
                if tj == group_tiles - 1:
                    finish_pass(gj, hfj, p2_of.pop(pj))

    nc.compile()
    return nc


def marshal_core_inputs(q, k, weights, core, m_loc=M_LOC, group_tiles=32):
    """Host-side layout marshalling for one core (permute/transpose/cast).

    Tile layout: 8 m's x 16 head-ranks per 128-col tile. Per group of
    128 m's: tiles 0..15 carry each m-octet's 16 largest-|w| heads
    (bf16 path, w in wblk); tiles 16..31 carry the 16 smallest (fp8
    path, w in the DoubleRow S blocks)."""
    f8 = ml_dtypes.float8_e4m3
    bf16 = ml_dtypes.bfloat16

    q_sh = np.asarray(q[0, core * m_loc:(core + 1) * m_loc])   # (256,H,D) bf16
    w_sh = np.asarray(weights[core * m_loc:(core + 1) * m_loc, 0, :]).astype(np.float32)
    order = np.argsort(-np.abs(w_sh), axis=1)                  # (256,H)
    w_sorted = np.take_along_axis(w_sh, order, 1)              # (256,H)
    q_sorted = np.take_along_axis(q_sh, order[:, :, None], 1)  # (256,H,D)

    kT = np.ascontiguousarray(np.asarray(k[0]).T)              # (128,N)

    gp = 4 * group_tiles  # 128
    qT = np.empty((128, MH), dtype=bf16)
    wblk = np.zeros((128, N_TILES * gp), dtype=bf16)
    S = np.zeros((128, 2 * 8 * 2 * 128), dtype=f8)
    rows = np.arange(128)
    for g in range(2):
        for t in range(32):
            tg = g * 32 + t
            o = t % 16                              # m-octet within group
            r0 = 0 if t < 16 else 16                # head-rank offset
            ms = 128 * g + 8 * o + np.arange(8)     # the 8 m's (global)
            blk = q_sorted[ms][:, r0:r0 + 16]       # (8,16,D)
            qT[:, tg * 128:(tg + 1) * 128] = blk.reshape(128, D).T
            wv = w_sorted[ms][:, r0:r0 + 16].reshape(128)   # p = 16*mi + r
            cols = np.repeat(8 * o + np.arange(8), 16)      # local m per p
            if t < 16:
                wblk[rows, tg * gp + cols] = wv.astype(bf16)
            else:
                i, j = divmod(t - 16, 2)
                S[rows, ((g * 8 + i) * 2 + j) * 128 + cols] = wv.astype(f8)

    return {"qT": qT, "kT": kT, "wblk": wblk, "S": S}


_NC_CACHE = {}


def _get_nc():
    if "nc" not in _NC_CACHE:
        _NC_CACHE["nc"] = build_nc()
    return _NC_CACHE["nc"]


def kernel(q, k, weights):
    nc = _get_nc()
    in_maps = [marshal_core_inputs(q, k, weights, c) for c in range(N_CORES)]
    res = run_bass_kernel_spmd(nc, in_maps, list(range(N_CORES)))
    out = np.concatenate([res.results[c]["o"] for c in range(N_CORES)], axis=0)
    return out[None]  # (1, M, N) fp32

